# revision 7
# baseline (speedup 1.0000x reference)
"""Trainium2 Bass kernel for nn_NonLocalDenoiser (LIDIA Aggregation0, top-1 self
neighbor): weighted patch fold -> normalize -> unfold, per pseudo-frame.

Shapes (hardcoded): x (2, 24336, 14, 75), nlDists (28, 24336, 14),
nlInds (28, 24336, 14, 3), H=W=160, PS=5, C=3.

Sharding: t=28 frames, each split into top/bottom half-slabs (82 input patch
rows with 4-row halo, 78 output rows); bottom slabs are row+dy flipped so all
56 tasks are identical. 7 tasks per core across 8 cores.

Device pipeline per task:
  - one contiguous DMA loads the whole canvas: 75 pre-padded feature planes,
    a zeroed w-plane slot, and the dist plane (77 blocks of 160 cols)
  - ACT: w = exp(-dist) into the single w plane (156 cols)
  - DVE/Pool: feature planes *= w (broadcast over planes, split 30/45)
  - PE: fold = 2x25 float32r matmuls with shifted-identity weights
    accumulating the (y, {c0,c1,c2,wimg}, x) image canvas in PSUM
  - DVE: rimg = 1/wimg; nimg = img * rimg  (PSUM -> SBUF)
  - DMA unfold: 5 strided reads (one per dy) of nimg -> HBM
"""
import numpy as np

PS, C, NH, W = 5, 3, 156, 160
RIN, ROUT, HORF, VF = 82, 78, 14, 75
NT = 7            # tasks per core
NCORES = 8
T = 28            # pseudo-frames
NPATCH = NH * NH
NBLK = VF + 2     # 75 feature planes + w plane + dist plane
WOFF = VF * W     # w plane offset (cols)
DOFF = (VF + 1) * W  # dist plane offset (cols)
PITCH = NBLK * W  # 12320 floats per partition

# v index permutation for bottom (row-flipped) tasks: (c,dy,dx) -> (c,4-dy,dx)
VPERM = np.array([c * 25 + (4 - dy) * 5 + dx
                  for c in range(C) for dy in range(PS) for dx in range(PS)])

LAST_EXEC_NS = None


def _build_program(loop_reps=1, do_out=True, do_mm=True, do_tt=True):
    import contextlib
    import concourse.bass as bass
    import concourse.bacc as bacc
    import concourse.mybir as mybir
    import concourse.tile as tile

    f32 = mybir.dt.float32
    f32r = mybir.dt.float32r
    nc = bacc.Bacc(None)
    XS = nc.declare_dram_parameter("xs", [NT, RIN, PITCH], f32, isOutput=False)
    OUT = nc.declare_dram_parameter("out", [NT, PS, ROUT, PS, NH, C], f32,
                                    isOutput=True)
    M = RIN + 4  # img rows per slab (86)

    NDVE = 55  # feature planes multiplied on DVE (rest on Pool)

    with tile.TileContext(nc) as tc:
        with tc.tile_pool(name="const", bufs=1) as cpool, \
             tc.tile_pool(name="xsp", bufs=3) as xpool, \
             tc.tile_pool(name="im", bufs=2) as ipool, \
             tc.tile_pool(name="ps", bufs=2, space="PSUM") as ppool:
            # 5 shifted identities: ids_dy[hi, y] = 1 iff y == hi + dy
            ids = cpool.tile([RIN, PS * M], f32)
            nc.gpsimd.memset(ids[:], 0.0)
            for dy in range(PS):
                sl = ids[:, dy * M:(dy + 1) * M]
                nc.gpsimd.affine_select(
                    out=sl, in_=sl, pattern=[[-1, M]],
                    compare_op=mybir.AluOpType.not_equal, fill=1.0,
                    base=dy, channel_multiplier=1)

            def norm_unfold(j, psA, psB):
                # nimg layout: [y, x*C + c] (channel-interleaved) so the
                # unfold DMA reads (wi, c) as one 468-elem contiguous run
                rimg = ipool.tile([M, W], f32, tag="rimg")
                nimg = ipool.tile([M, C * W], f32, tag="nimg")
                nc.vector.reciprocal(out=rimg[:], in_=psB[:, W:2 * W])
                nimg3 = nimg[:].rearrange("p (q c) -> p c q", c=C)
                for c, (pt, off) in enumerate(((psA, 0), (psA, W),
                                               (psB, 0))):
                    nc.vector.tensor_tensor(
                        out=nimg3[:, c, :], in0=pt[:, off:off + W],
                        in1=rimg[:], op=mybir.AluOpType.mult)
                # unfold: out[dy, hi, dx, wi, c] = nimg[hi+dy, (wi+dx)*C+c]
                npitch = nimg[:].ap[0][0]
                if do_out:
                    for dy in range(PS):
                        s = nimg[dy:dy + ROUT, :]
                        src = bass.AP(s.tensor, s.offset,
                                      [[npitch, ROUT], [C, PS],
                                       [1, C * NH]])
                        nc.scalar.dma_start(out=OUT[j, dy], in_=src)

            loop_cm = (tc.For_i(0, loop_reps) if loop_reps > 1
                       else contextlib.nullcontext())
            with loop_cm:
              prev = None
              for j in range(NT):
                xs_t = xpool.tile([RIN, PITCH], f32, tag="xs")
                full = xs_t[:]
                pitch = full.ap[0][0]
                # whole canvas in one contiguous DMA (features + zeroed
                # w slot + dists, pads shipped from the host)
                nc.sync.dma_start(out=full, in_=XS[j])
                # w plane = exp(-d)
                nc.scalar.activation(
                    out=xs_t[:, WOFF + 4:WOFF + W],
                    in_=xs_t[:, DOFF + 4:DOFF + W],
                    func=mybir.ActivationFunctionType.Exp, scale=-1.0)
                # feature planes *= w (w broadcast over planes); split
                # across DVE and Pool to balance engine time
                w_ap = xs_t[:, WOFF:WOFF + W]
                if do_tt:
                    for eng, p0, np_ in ((nc.vector, 0, NDVE),
                                         (nc.gpsimd, NDVE, VF - NDVE)):
                        pl = xs_t[:, p0 * W:(p0 + np_) * W].rearrange(
                            "p (v q) -> p v q", q=W)
                        eng.tensor_tensor(
                            out=pl, in0=pl,
                            in1=w_ap.unsqueeze(1).to_broadcast(
                                [RIN, np_, W]),
                            op=mybir.AluOpType.mult)

                # fold: psA = (c0,c1) image canvas, psB = (c2, wimg)
                psA = ppool.tile([M, 2 * W], f32, tag="psA", space="PSUM")
                psB = ppool.tile([M, 2 * W], f32, tag="psB", space="PSUM")
                nv0 = 25 if do_mm else 1
                for dy in range(PS):
                    lhsT = ids[:, dy * M:(dy + 1) * M]
                    for dx in range(PS):
                        v0 = dy * PS + dx
                        if v0 >= nv0:
                            continue
                        for ps_t, off, st2 in (
                                (psA, v0 * W, 25 * W),
                                (psB, (50 + v0) * W, (25 - v0) * W)):
                            rs = bass.AP(
                                full.tensor,
                                full.offset + off + 4 - dx,
                                [[pitch, RIN], [st2, 2], [1, W]],
                            )
                            nc.tensor.matmul(out=ps_t[:], lhsT=lhsT,
                                             rhs=rs, start=(v0 == 0),
                                             stop=(v0 == nv0 - 1))

                # normalize+unfold for the PREVIOUS task so the DVE never
                # stalls waiting on this task's matmuls
                if prev is not None:
                    norm_unfold(j - 1, *prev)
                prev = (psA, psB)
              norm_unfold(NT - 1, *prev)
    nc.finalize()
    return nc


def _host_prep(x, nlDists):
    # xt[tau, hi, v, wi] = x[i, hi*156+wi, f, v],  tau = i*14+f
    xt = np.ascontiguousarray(
        x.reshape(2, NH, NH, HORF, VF).transpose(0, 3, 1, 4, 2)
    ).reshape(T, NH, VF, NH)
    d6 = np.ascontiguousarray(nlDists[:, :, 0]).reshape(T, NH, NH)
    # canvas: 75 feature planes as [4 zero cols | data], zeroed w slot,
    # dist plane as [4 zero cols | data] -> one contiguous DMA per task
    XSa = np.zeros((2 * T, RIN, NBLK, W), np.float32)
    XSa[0::2, :, :VF, 4:] = xt[:, :RIN]
    XSa[1::2, :, :VF, 4:] = xt[:, NH - RIN:][:, ::-1][:, :, VPERM, :]
    XSa[0::2, :, VF + 1, 4:] = d6[:, :RIN]
    XSa[1::2, :, VF + 1, 4:] = d6[:, NH - RIN:][:, ::-1]
    return XSa.reshape(NCORES, NT, RIN, PITCH)


def _make_in_maps(x, nlDists):
    XSa = _host_prep(x, nlDists)
    return [{"xs": XSa[c]} for c in range(NCORES)]


def _host_post(OUTa):
    # OUTa: (8, 7, 5, 78, 5, 156, 3) -> (2, 24336, 14, 75)
    O = OUTa.reshape(2 * T, PS, ROUT, PS, NH, C)
    top, bot = O[0::2], O[1::2]
    out6 = np.empty((T, NH, NH, C, PS, PS), np.float32)
    # [tau, dy, hi, dx, wi, c] -> [tau, hi, wi, c, dy, dx]
    out6[:, :ROUT] = top.transpose(0, 2, 4, 5, 1, 3)
    out6[:, ROUT:] = bot[:, ::-1].transpose(0, 2, 4, 5, 1, 3)[:, ::-1]
    out_flat = out6.reshape(T, NPATCH, VF)
    final = out_flat.reshape(2, HORF, VF, NPATCH).transpose(0, 3, 1, 2)
    return np.ascontiguousarray(final)


def _is_self_inds(nlInds):
    k0 = np.asarray(nlInds)[:, :, 0, :]
    j = np.arange(NPATCH)
    return (bool((k0[:, :, 0] == np.arange(T, dtype=k0.dtype)[:, None]).all())
            and bool((k0[:, :, 1] == (j // NH).astype(k0.dtype)).all())
            and bool((k0[:, :, 2] == (j % NH).astype(k0.dtype)).all()))


def _numpy_fallback(x, nlDists, nlInds, H, Wp):
    images, patches, hor_f, ver_f = x.shape
    t = images * hor_f
    N = t * patches
    xr = np.transpose(x, (0, 2, 3, 1)).reshape(t, ver_f, patches)
    pat = np.transpose(xr, (0, 2, 1)).reshape(N, C, PS, PS)
    w = np.exp(-nlDists[:, :, 0].reshape(N))
    inds = nlInds[:, :, 0, :].reshape(N, 3)
    ti, hi, wi = inds[:, 0], inds[:, 1], inds[:, 2]
    d = np.arange(PS)
    sidx = (ti[:, None, None] * (H * Wp)
            + (hi[:, None, None] + d[None, :, None]) * Wp
            + (wi[:, None, None] + d[None, None, :])).reshape(-1)
    vals = (w[:, None, None, None] * pat).transpose(0, 2, 3, 1).reshape(-1, C)
    img = np.zeros((t * H * Wp, C), x.dtype)
    np.add.at(img, sidx, vals)
    wimg = np.zeros((t * H * Wp,), x.dtype)
    np.add.at(wimg, sidx, np.repeat(w, PS * PS))
    img = img / wimg[:, None]
    out_pat = img[sidx].reshape(N, PS, PS, C).transpose(0, 3, 1, 2)
    out = out_pat.reshape(t, patches, ver_f)
    return np.ascontiguousarray(
        out.reshape(images, hor_f, ver_f, patches).transpose(0, 3, 1, 2))


def kernel(x, nlDists, nlInds, pixels_h, pixels_w):
    global LAST_EXEC_NS
    import os
    x = np.asarray(x, np.float32)
    nlDists = np.asarray(nlDists, np.float32)
    if (x.shape != (2, NPATCH, HORF, VF) or int(pixels_h) != 160
            or int(pixels_w) != 160 or not _is_self_inds(nlInds)):
        return _numpy_fallback(np.asarray(x), np.asarray(nlDists),
                               np.asarray(nlInds), int(pixels_h), int(pixels_w))

    from concourse.bass_utils import run_bass_kernel_spmd
    in_maps = _make_in_maps(x, nlDists)
    nc = _build_program()
    trace = bool(os.environ.get("BASS_KERNEL_PROFILE"))
    res = run_bass_kernel_spmd(nc, in_maps, list(range(NCORES)), trace=trace)
    LAST_EXEC_NS = res.exec_time_ns
    OUTa = np.stack([np.asarray(res.results[c]["out"], np.float32)
                     for c in range(NCORES)])
    return _host_post(OUTa)


# revision 26
# speedup vs baseline: 1.2116x; 1.2116x over previous
"""Trainium2 Bass kernel for nn_NonLocalDenoiser (LIDIA Aggregation0, top-1 self
neighbor): weighted patch fold -> normalize -> unfold, per pseudo-frame.

Shapes (hardcoded): x (2, 24336, 14, 75), nlDists (28, 24336, 14),
nlInds (28, 24336, 14, 3), H=W=160, PS=5, C=3.

Sharding: t=28 frames, each split into top/bottom half-slabs (82 input patch
rows with 4-row halo, 78 output rows); bottom slabs are row+dy flipped so all
56 tasks are identical. 7 tasks per core across 8 cores.

Device pipeline per task:
  - one contiguous DMA loads the whole canvas: 75 pre-padded feature planes,
    a zeroed w-plane slot, and the dist plane (77 blocks of 160 cols)
  - ACT: w = exp(-dist) into the single w plane (156 cols)
  - DVE/Pool: feature planes *= w (broadcast over planes, split 30/45)
  - PE: fold = 2x25 float32r matmuls with shifted-identity weights
    accumulating the (y, {c0,c1,c2,wimg}, x) image canvas in PSUM
  - DVE: rimg = 1/wimg; nimg = img * rimg  (PSUM -> SBUF)
  - DMA unfold: 5 strided reads (one per dy) of nimg -> HBM
"""
import numpy as np

PS, C, NH, W = 5, 3, 156, 160
RIN, ROUT, HORF, VF = 82, 78, 14, 75
NT = 7            # tasks per core
NCORES = 8
T = 28            # pseudo-frames
NPATCH = NH * NH
NBLK = VF + 2     # 75 feature planes + w plane + dist plane
WOFF = VF * W     # w plane offset (cols)
DOFF = (VF + 1) * W  # dist plane offset (cols)
PITCH = NBLK * W + 4  # 12324 floats per partition (4-col zero tail)

# v index permutation for bottom (row-flipped) tasks: (c,dy,dx) -> (c,4-dy,dx)
VPERM = np.array([c * 25 + (4 - dy) * 5 + dx
                  for c in range(C) for dy in range(PS) for dx in range(PS)])

LAST_EXEC_NS = None


def _build_program(loop_reps=1, do_out=True, do_mm=True, do_tt=True,
                   ndve=None, presum=True, do_in=True, do_rest=True,
                   staggered=False):
    import contextlib
    import concourse.bass as bass
    import concourse.bacc as bacc
    import concourse.mybir as mybir
    import concourse.tile as tile

    f32 = mybir.dt.float32
    f32r = mybir.dt.float32r
    nc = bacc.Bacc(None)
    XS = nc.declare_dram_parameter("xs", [NT, RIN, PITCH], f32, isOutput=False)
    OUT = nc.declare_dram_parameter("out", [NT, PS, ROUT, PS, NH, C], f32,
                                    isOutput=True)
    M = RIN + 4  # img rows per slab (86)

    # planes multiplied on DVE (rest on Pool); with presum, align to the
    # presum group split (DVE groups 0-9 read planes 0-49)
    NDVE = (50 if presum else 55) if ndve is None else ndve

    with tile.TileContext(nc) as tc:
        with tc.tile_pool(name="const", bufs=1) as cpool, \
             tc.tile_pool(name="xsp", bufs=3) as xpool, \
             tc.tile_pool(name="im", bufs=2) as ipool, \
             tc.tile_pool(name="ps", bufs=2, space="PSUM") as ppool:
            # 5 shifted identities: ids_dy[hi, y] = 1 iff y == hi + dy
            ids = cpool.tile([RIN, PS * M], f32)
            nc.gpsimd.memset(ids[:], 0.0)
            for dy in range(PS):
                sl = ids[:, dy * M:(dy + 1) * M]
                nc.gpsimd.affine_select(
                    out=sl, in_=sl, pattern=[[-1, M]],
                    compare_op=mybir.AluOpType.not_equal, fill=1.0,
                    base=dy, channel_multiplier=1)

            def norm_unfold(j, psA, psB):
                # nimg layout: [y, x*C + c] (channel-interleaved) so the
                # dx-expansion reads (wi, c) as one 468-elem contiguous run
                rimg = ipool.tile([M, W], f32, tag="rimg")
                nimg = ipool.tile([M, C * W], f32, tag="nimg")
                nc.vector.reciprocal(out=rimg[:], in_=psB[:, W:2 * W])
                nimg3 = nimg[:].rearrange("p (q c) -> p c q", c=C)
                for c, (pt, off) in enumerate(((psA, 0), (psA, W),
                                               (psB, 0))):
                    nc.vector.tensor_tensor(
                        out=nimg3[:, c, :], in0=pt[:, off:off + W],
                        in1=rimg[:], op=mybir.AluOpType.mult)
                # unfold: out[dy, hi, dx, wi, c] = nimg[hi+dy, (wi+dx)*C+c]
                npitch = nimg[:].ap[0][0]
                if do_out:
                    for dy in range(PS):
                        s = nimg[dy:dy + ROUT, :]
                        src = bass.AP(s.tensor, s.offset,
                                      [[npitch, ROUT], [C, PS],
                                       [1, C * NH]])
                        nc.scalar.dma_start(out=OUT[j, dy], in_=src)

            loop_cm = (tc.For_i(0, loop_reps, staggered_reset=staggered)
                       if loop_reps > 1 else contextlib.nullcontext())
            with loop_cm:
              prev = None
              for j in range(NT):
                xs_t = xpool.tile([RIN, PITCH], f32, tag="xs")
                full = xs_t[:]
                pitch = full.ap[0][0]
                # whole canvas in one contiguous DMA (features + zeroed
                # w slot + dists, pads shipped from the host)
                if do_in:
                    nc.sync.dma_start(out=full, in_=XS[j])
                if not do_rest:
                    if not do_in and not do_out:
                        z = ipool.tile([M, 4], f32, tag="rimg")
                        nc.gpsimd.memset(z[:], 0.0)
                    if do_out:
                        nimg = ipool.tile([M, C * W], f32, tag="nimg")
                        nc.vector.memset(nimg[:], 1.0)
                        npitch = nimg[:].ap[0][0]
                        for dy in range(PS):
                            s = nimg[dy:dy + ROUT, :]
                            src = bass.AP(s.tensor, s.offset,
                                          [[npitch, ROUT], [C, PS],
                                           [1, C * NH]])
                            nc.scalar.dma_start(out=OUT[j, dy], in_=src)
                    continue
                # w plane = exp(-d)
                nc.scalar.activation(
                    out=xs_t[:, WOFF + 4:WOFF + W],
                    in_=xs_t[:, DOFF + 4:DOFF + W],
                    func=mybir.ActivationFunctionType.Exp, scale=-1.0)
                # feature planes *= w (w broadcast over planes); split
                # across DVE and Pool to balance engine time
                w_ap = xs_t[:, WOFF:WOFF + W]
                if do_tt:
                    for eng, p0, np_ in ((nc.vector, 0, NDVE),
                                         (nc.gpsimd, NDVE, VF - NDVE)):
                        if not np_:
                            continue
                        pl = xs_t[:, p0 * W:(p0 + np_) * W].rearrange(
                            "p (v q) -> p v q", q=W)
                        eng.tensor_tensor(
                            out=pl, in0=pl,
                            in1=w_ap.unsqueeze(1).to_broadcast(
                                [RIN, np_, W]),
                            op=mybir.AluOpType.mult)

                def cav(off, dims):
                    return bass.AP(full.tensor, full.offset + off, dims)

                if presum and do_tt:
                    # dx-presum on DVE/Pool: S'[u] = sum_{dx=1..4}
                    # wv_dx[u+1-dx], stored in-place at the dx=1 plane
                    # slots (planes 5g+1), cols [3, 163).
                    for eng, g0, ng in ((nc.vector, 0, 10),
                                        (nc.gpsimd, 10, 5)):
                        base = (5 * g0 + 1) * W + 3
                        o = cav(base, [[pitch, RIN], [5 * W, ng], [1, W]])
                        for m in range(1, 4):
                            i1 = cav(base + m * W - m,
                                     [[pitch, RIN], [5 * W, ng], [1, W]])
                            eng.tensor_tensor(out=o, in0=o, in1=i1,
                                              op=mybir.AluOpType.add)
                    # Sw'[u] = w[u]+w[u-1]+w[u-2]+w[u-3] into the (spent)
                    # dist plane, cols [3, 163)
                    so = cav(DOFF + 3, [[pitch, RIN], [1, W]])
                    nc.gpsimd.tensor_tensor(
                        out=so, in0=cav(WOFF + 3, [[pitch, RIN], [1, W]]),
                        in1=cav(WOFF + 2, [[pitch, RIN], [1, W]]),
                        op=mybir.AluOpType.add)
                    for m in (2, 3):
                        nc.gpsimd.tensor_tensor(
                            out=so, in0=so,
                            in1=cav(WOFF + 3 - m, [[pitch, RIN], [1, W]]),
                            op=mybir.AluOpType.add)

                # fold: psA = (c0,c1) image canvas, psB = (c2, wimg)
                psA = ppool.tile([M, 2 * W], f32, tag="psA", space="PSUM")
                psB = ppool.tile([M, 2 * W], f32, tag="psB", space="PSUM")
                if presum:
                    # per dy: dx=0 raw planes (offset +4) and S'/Sw'
                    # presummed planes (offset +3, dx=1 slots)
                    for dy in range(PS if do_mm else 1):
                        lhsT = ids[:, dy * M:(dy + 1) * M]
                        mms = (
                            (psA, (5 * dy) * W + 4, 25 * W),
                            (psA, (5 * dy + 1) * W + 3, 25 * W),
                            (psB, (50 + 5 * dy) * W + 4, (25 - 5 * dy) * W),
                            (psB, (51 + 5 * dy) * W + 3, (25 - 5 * dy) * W),
                        )
                        last_dy = 4 if do_mm else 0
                        for k, (ps_t, off, st2) in enumerate(mms):
                            rs = cav(off, [[pitch, RIN], [st2, 2], [1, W]])
                            nc.tensor.matmul(
                                out=ps_t[:], lhsT=lhsT, rhs=rs,
                                start=(dy == 0 and k in (0, 2)),
                                stop=(dy == last_dy and k in (1, 3)))
                else:
                    nv0 = 25 if do_mm else 1
                    for dy in range(PS):
                        lhsT = ids[:, dy * M:(dy + 1) * M]
                        for dx in range(PS):
                            v0 = dy * PS + dx
                            if v0 >= nv0:
                                continue
                            for ps_t, off, st2 in (
                                    (psA, v0 * W, 25 * W),
                                    (psB, (50 + v0) * W, (25 - v0) * W)):
                                rs = cav(off + 4 - dx,
                                         [[pitch, RIN], [st2, 2], [1, W]])
                                nc.tensor.matmul(out=ps_t[:], lhsT=lhsT,
                                                 rhs=rs, start=(v0 == 0),
                                                 stop=(v0 == nv0 - 1))

                # normalize+unfold for the PREVIOUS task so the DVE never
                # stalls waiting on this task's matmuls
                if prev is not None:
                    norm_unfold(j - 1, *prev)
                prev = (psA, psB)
              if prev is not None:
                  norm_unfold(NT - 1, *prev)
    nc.finalize()
    return nc


def _host_prep(x, nlDists):
    # xt[tau, hi, v, wi] = x[i, hi*156+wi, f, v],  tau = i*14+f
    xt = np.ascontiguousarray(
        x.reshape(2, NH, NH, HORF, VF).transpose(0, 3, 1, 4, 2)
    ).reshape(T, NH, VF, NH)
    d6 = np.ascontiguousarray(nlDists[:, :, 0]).reshape(T, NH, NH)
    # canvas: 75 feature planes as [4 zero cols | data], zeroed w slot,
    # dist plane as [4 zero cols | data], 4-col zero tail -> one
    # contiguous DMA per task
    XSa = np.zeros((2 * T, RIN, PITCH), np.float32)
    XSb = XSa[:, :, :NBLK * W].reshape(2 * T, RIN, NBLK, W)
    XSb[0::2, :, :VF, 4:] = xt[:, :RIN]
    XSb[1::2, :, :VF, 4:] = xt[:, NH - RIN:][:, ::-1][:, :, VPERM, :]
    XSb[0::2, :, VF + 1, 4:] = d6[:, :RIN]
    XSb[1::2, :, VF + 1, 4:] = d6[:, NH - RIN:][:, ::-1]
    return XSa.reshape(NCORES, NT, RIN, PITCH)


def _make_in_maps(x, nlDists):
    XSa = _host_prep(x, nlDists)
    return [{"xs": XSa[c]} for c in range(NCORES)]


def _host_post(OUTa):
    # OUTa: (8, 7, 5, 78, 5, 156, 3) -> (2, 24336, 14, 75)
    O = OUTa.reshape(2 * T, PS, ROUT, PS, NH, C)
    top, bot = O[0::2], O[1::2]
    out6 = np.empty((T, NH, NH, C, PS, PS), np.float32)
    # [tau, dy, hi, dx, wi, c] -> [tau, hi, wi, c, dy, dx]
    out6[:, :ROUT] = top.transpose(0, 2, 4, 5, 1, 3)
    out6[:, ROUT:] = bot[:, ::-1].transpose(0, 2, 4, 5, 1, 3)[:, ::-1]
    out_flat = out6.reshape(T, NPATCH, VF)
    final = out_flat.reshape(2, HORF, VF, NPATCH).transpose(0, 3, 1, 2)
    return np.ascontiguousarray(final)


def _is_self_inds(nlInds):
    k0 = np.asarray(nlInds)[:, :, 0, :]
    j = np.arange(NPATCH)
    return (bool((k0[:, :, 0] == np.arange(T, dtype=k0.dtype)[:, None]).all())
            and bool((k0[:, :, 1] == (j // NH).astype(k0.dtype)).all())
            and bool((k0[:, :, 2] == (j % NH).astype(k0.dtype)).all()))


def _numpy_fallback(x, nlDists, nlInds, H, Wp):
    images, patches, hor_f, ver_f = x.shape
    t = images * hor_f
    N = t * patches
    xr = np.transpose(x, (0, 2, 3, 1)).reshape(t, ver_f, patches)
    pat = np.transpose(xr, (0, 2, 1)).reshape(N, C, PS, PS)
    w = np.exp(-nlDists[:, :, 0].reshape(N))
    inds = nlInds[:, :, 0, :].reshape(N, 3)
    ti, hi, wi = inds[:, 0], inds[:, 1], inds[:, 2]
    d = np.arange(PS)
    sidx = (ti[:, None, None] * (H * Wp)
            + (hi[:, None, None] + d[None, :, None]) * Wp
            + (wi[:, None, None] + d[None, None, :])).reshape(-1)
    vals = (w[:, None, None, None] * pat).transpose(0, 2, 3, 1).reshape(-1, C)
    img = np.zeros((t * H * Wp, C), x.dtype)
    np.add.at(img, sidx, vals)
    wimg = np.zeros((t * H * Wp,), x.dtype)
    np.add.at(wimg, sidx, np.repeat(w, PS * PS))
    img = img / wimg[:, None]
    out_pat = img[sidx].reshape(N, PS, PS, C).transpose(0, 3, 1, 2)
    out = out_pat.reshape(t, patches, ver_f)
    return np.ascontiguousarray(
        out.reshape(images, hor_f, ver_f, patches).transpose(0, 3, 1, 2))


def kernel(x, nlDists, nlInds, pixels_h, pixels_w):
    global LAST_EXEC_NS
    import os
    x = np.asarray(x, np.float32)
    nlDists = np.asarray(nlDists, np.float32)
    if (x.shape != (2, NPATCH, HORF, VF) or int(pixels_h) != 160
            or int(pixels_w) != 160 or not _is_self_inds(nlInds)):
        return _numpy_fallback(np.asarray(x), np.asarray(nlDists),
                               np.asarray(nlInds), int(pixels_h), int(pixels_w))

    from concourse.bass_utils import run_bass_kernel_spmd
    in_maps = _make_in_maps(x, nlDists)
    nc = _build_program()
    trace = bool(os.environ.get("BASS_KERNEL_PROFILE"))
    res = run_bass_kernel_spmd(nc, in_maps, list(range(NCORES)), trace=trace)
    LAST_EXEC_NS = res.exec_time_ns
    OUTa = np.stack([np.asarray(res.results[c]["out"], np.float32)
                     for c in range(NCORES)])
    return _host_post(OUTa)
